# revision 55
# baseline (speedup 1.0000x reference)
"""Trainium2 Bass kernel for nn_Bipartite: bipartite GNN edge scoring.

reference computation:
    src = nf_task @ W1[:, :D].T          [T, D]
    dst = nf_agent @ W1[:, D:].T         [A, D]
    h[t,a,:] = BN1(src[t] + dst[a]); h = lrelu(h)
    s[t,a] = h @ W2[0];  s = BN2(s); s[finished] = -inf;  out = s.T  [A, T]

Key identities used (avoid materializing [T, A, D]):
  - BN1 batch stats decompose: mean = mean_t(src) + mean_a(dst),
    var = var_t(src) + var_a(dst)   (h is a broadcast sum).
  - lrelu(x) = 0.01*x + 0.99*relu(x), and the 0.01*x part of the score is
    rank-1:  sum_d w2_d * x[t,a,d] = pdot[t] + qdot[a].
  - so  s_raw[t,a] = 0.99 * sum_d w2_d * relu(P'[d,t] + Q'[d,a])
                   + 0.01 * (pdot[t] + qdot[a])
    with P' = a1*src.T + b1 (BN1 affine folded task-side), Q' = a1*dst.T.

Sharding: tasks (T axis) split across 8 cores; BN1 stats computed
replicated on every core (cheap); BN2 batch stats via a tiny AllGather of
per-core (sum, sumsq); the -inf mask and final affine are applied on-device.
Per-task inner loop: one fused add+relu op (DVE or ACT) produces
r = relu(Q' + P'[:,t]) [D=128 part, A=512 free]; four matmuls with r-chunks
stationary x w2 column moving write score columns directly into PSUM in
[agent, task] (output-transposed) layout.
"""
import os
import sys

_REPO = "/opt/trn_rl_repo"
if not os.path.isdir(_REPO):
    _REPO = "/root/.axon_site/_ro/trn_rl_repo"
if _REPO in sys.path:
    sys.path.remove(_REPO)
sys.path.insert(0, _REPO)

import numpy as np
import concourse.bass as bass  # noqa: F401  (registers engines)
import concourse.bacc as bacc
import concourse.tile as tile
import concourse.mybir as mybir
from concourse import bass_utils

T, A, D = 1024, 512, 128
NCORES = 8
TLOC = T // NCORES  # 128 tasks per core
EPS = 1e-5
NEG = 0.01  # LeakyReLU slope
F32 = mybir.dt.float32
BF16 = mybir.dt.bfloat16
AX = mybir.AxisListType
OP = mybir.AluOpType
AF = mybir.ActivationFunctionType

# r-path dtype: fp16 halves PE stationary-load time and doubles DVE rate
# vs fp32 (same speed as bf16 but 8x better mantissa; values are < ~10 so
# fp16 range is ample).
F16 = mybir.dt.float16
R_DTYPE = F16
ACT_EVERY = 4  # every ACT_EVERY-th task runs on ScalarE, rest on VectorE

_CACHE: dict = {}


def _inv_sqrt(nc, pool, v, name):
    """[128,1] column: 1/sqrt(v) = exp(-0.5*ln(v)), one Newton step.

    Uses Ln+Exp (one ACT table set, shared with Relu/Square — avoids the
    ~2.6us extra table load that Sqrt's set would cost), then a
    Newton-Raphson step y*(1.5 - 0.5*v*y^2) on the accurate DVE ops.
    """
    # quake seed: bits = MAGIC - (bits(v) >> 1), done as
    # ((v>>1) xor 0xffffffff) - (0xffffffff - MAGIC)   (all uint32 ALU)
    U32 = mybir.dt.uint32
    MAGIC = 0x5F3759DF
    K2 = 0xFFFFFFFF - MAGIC
    tA = pool.tile([D, 1], F32, tag=f"{name}_tA")
    nc.vector.tensor_scalar(tA[:].bitcast(U32), v[:].bitcast(U32), 1, None,
                            OP.logical_shift_right)
    tB = pool.tile([D, 1], F32, tag=f"{name}_tB")
    nc.vector.tensor_scalar(tB[:].bitcast(U32), tA[:].bitcast(U32),
                            0xFFFFFFFF, None, OP.bitwise_xor)
    y = pool.tile([D, 1], F32, tag=f"{name}_y0")
    nc.vector.tensor_scalar(y[:].bitcast(U32), tB[:].bitcast(U32),
                            K2, None, OP.subtract)
    # three Newton-Raphson steps: y <- y*(1.5 - 0.5*v*y^2)
    for it in range(2):
        y2 = pool.tile([D, 1], F32, tag=f"{name}_y2{it}")
        nc.vector.tensor_tensor(y2[:], y[:], y[:], op=OP.mult)
        vy2 = pool.tile([D, 1], F32, tag=f"{name}_vy2{it}")
        nc.vector.tensor_tensor(vy2[:], v[:], y2[:], op=OP.mult)
        h = pool.tile([D, 1], F32, tag=f"{name}_h{it}")
        nc.vector.tensor_scalar(h[:], vy2[:], -0.5, 1.5, OP.mult, OP.add)
        yn = pool.tile([D, 1], F32, tag=f"{name}_yn{it}")
        nc.vector.tensor_tensor(yn[:], y[:], h[:], op=OP.mult)
        y = yn
    return y


def _build(r_dtype=R_DTYPE, act_every=ACT_EVERY, dbg=False, single=False):
    """single=True: 1-core variant with the collective replaced by a DRAM
    bounce copy — numerically wrong stats but timing-shape identical; used
    for TimelineSim cost-model benchmarking (which is single-core only)."""
    nc = bacc.Bacc("TRN2", target_bir_lowering=False, debug=False,
                   enable_asserts=True, num_devices=1 if single else NCORES)

    # ---- kernel I/O (per core) ----
    taskT_d = nc.dram_tensor("taskT", [D, T], F16, kind="ExternalInput")
    agentT_d = nc.dram_tensor("agentT", [D, A], F16, kind="ExternalInput")
    w16_d = nc.dram_tensor("w16", [D, 2 * D + TLOC], F16, kind="ExternalInput")
    gbw_d = nc.dram_tensor("gbw", [D, 3], F32, kind="ExternalInput")
    meta_d = nc.dram_tensor("meta", [1, 2 + TLOC], F32, kind="ExternalInput")
    pol_d = nc.dram_tensor("policy", [A, TLOC], F32, kind="ExternalOutput")
    if dbg:
        dbg_racc0_d = nc.dram_tensor("dbg_racc0", [D, TLOC], F32, kind="ExternalOutput")
        dbg_spre0_d = nc.dram_tensor("dbg_spre0", [D, TLOC], F32, kind="ExternalOutput")
        dbg_ag_d = nc.dram_tensor("dbg_ag", [1, NCORES * 8], F32, kind="ExternalOutput")
        dbg_cols_d = nc.dram_tensor("dbg_cols", [D, 12], F32, kind="ExternalOutput")
        dbg_st_d = nc.dram_tensor("dbg_st", [D, 16], F32, kind="ExternalOutput")
        dbg_cc_d = nc.dram_tensor("dbg_cc", [1, 16], F32, kind="ExternalOutput")

    with tile.TileContext(nc) as tc:
        with tc.tile_pool(name="const", bufs=1) as cp, \
             tc.tile_pool(name="work", bufs=2) as wk, \
             tc.tile_pool(name="rbuf", bufs=10) as rb, \
             tc.tile_pool(name="proj_ps", bufs=3, space="PSUM") as pps, \
             tc.tile_pool(name="small_ps", bufs=1, space="PSUM") as sps, \
             tc.tile_pool(name="racc_ps", bufs=1, space="PSUM") as rps, \
             tc.tile_pool(name="dram", bufs=1, space="DRAM") as dp:

            # ---------- loads ----------
            tT = cp.tile([D, T], F16, tag="tT")
            for _q in range(2):
                nc.sync.dma_start(out=tT[:, _q * 512:(_q + 1) * 512],
                                  in_=taskT_d[:, _q * 512:(_q + 1) * 512])
            aT = cp.tile([D, A], F16, tag="aT")
            nc.sync.dma_start(out=aT[:], in_=agentT_d.ap())
            w16 = cp.tile([D, 2 * D + TLOC], F16, tag="w16")
            nc.sync.dma_start(out=w16[:], in_=w16_d.ap())
            wsrc, wdst, tTl = w16[:, 0:D], w16[:, D:2 * D], w16[:, 2 * D:]
            gbw = cp.tile([D, 3], F32, tag="gbw")
            nc.sync.dma_start(out=gbw[:], in_=gbw_d.ap())
            g1, b1, w2 = gbw[:, 0:1], gbw[:, 1:2], gbw[:, 2:3]
            meta = cp.tile([1, 2 + TLOC], F32, tag="meta")
            nc.sync.dma_start(out=meta[:], in_=meta_d.ap())
            g2b2 = meta[:, 0:2]
            mskr = meta[:, 2:]

            # mask broadcast to all partitions (needed only at the end;
            # no data deps so it can schedule any time)
            msk_b = cp.tile([D, TLOC], F32, tag="msk_b")
            nc.gpsimd.partition_broadcast(msk_b[:], mskr)

            # ---------- projections (PE) + BN1 stats ----------
            # PT[j, t] = sum_c W1src[j, c] * taskT[c, t]  (full T for stats)
            sumcols = cp.tile([D, 6], F32, tag="sumcols")   # sums: PT0 PT1 QT
            sqcols = cp.tile([D, 6], F32, tag="sqcols")     # sqsums

            sq_scr = wk.tile([D, 512], F32, tag="sq_scr")
            sq_scr2 = wk.tile([D, 512], F32, tag="sq_scr2")

            def stats_of(ps_tile, n, col):
                nc.vector.tensor_reduce(sumcols[:, col:col + 1], ps_tile[:, 0:n],
                                        axis=AX.X, op=OP.add)
                nc.scalar.activation(sq_scr2[:, 0:n], ps_tile[:, 0:n], AF.Square,
                                     accum_out=sqcols[:, col:col + 1])

            for half in range(2):
                pt = pps.tile([D, 512], F32, tag="proj")
                nc.tensor.matmul(pt[:], wsrc, tT[:, half * 512:(half + 1) * 512],
                                 start=True, stop=True)
                stats_of(pt, 512, half)
            qt = pps.tile([D, 512], F32, tag="proj")
            nc.tensor.matmul(qt[:], wdst, aT[:], start=True, stop=True)
            stats_of(qt, 512, 2)
            ptl = pps.tile([D, TLOC], F32, tag="proj", name="ptl")
            nc.tensor.matmul(ptl[:], wsrc, tTl, start=True, stop=True)

            # per-channel BN1 stats [D, 1]
            st = cp  # alias: small stat tiles live in const pool
            sumP = st.tile([D, 1], F32, tag="sumP")
            nc.vector.tensor_tensor(sumP[:], sumcols[:, 0:1], sumcols[:, 1:2], op=OP.add)
            meanP = st.tile([D, 1], F32, tag="meanP")
            nc.vector.tensor_scalar(meanP[:], sumP[:], 1.0 / T, None, OP.mult)
            meanQ = st.tile([D, 1], F32, tag="meanQ")
            nc.vector.tensor_scalar(meanQ[:], sumcols[:, 2:3], 1.0 / A, None, OP.mult)
            m1 = st.tile([D, 1], F32, tag="m1")
            nc.vector.tensor_tensor(m1[:], meanP[:], meanQ[:], op=OP.add)
            # E[P^2] + E[Q^2]
            sqP = st.tile([D, 1], F32, tag="sqP")
            nc.vector.tensor_tensor(sqP[:], sqcols[:, 0:1], sqcols[:, 1:2], op=OP.add)
            ex2 = st.tile([D, 1], F32, tag="ex2")
            nc.vector.tensor_scalar(ex2[:], sqP[:], 1.0 / T, None, OP.mult)
            ex2q = st.tile([D, 1], F32, tag="ex2q")
            nc.vector.tensor_scalar(ex2q[:], sqcols[:, 2:3], 1.0 / A, None, OP.mult)
            # var = ex2 - meanP^2 + ex2q - meanQ^2
            mP2 = st.tile([D, 1], F32, tag="mP2")
            nc.vector.tensor_tensor(mP2[:], meanP[:], meanP[:], op=OP.mult)
            mQ2 = st.tile([D, 1], F32, tag="mQ2")
            nc.vector.tensor_tensor(mQ2[:], meanQ[:], meanQ[:], op=OP.mult)
            v_a = st.tile([D, 1], F32, tag="v_a")
            nc.vector.tensor_tensor(v_a[:], ex2[:], mP2[:], op=OP.subtract)
            v_b = st.tile([D, 1], F32, tag="v_b")
            nc.vector.tensor_tensor(v_b[:], ex2q[:], mQ2[:], op=OP.subtract)
            var1 = st.tile([D, 1], F32, tag="var1")
            nc.vector.tensor_tensor(var1[:], v_a[:], v_b[:], op=OP.add)
            vpe1 = st.tile([D, 1], F32, tag="vpe1")
            nc.vector.tensor_scalar(vpe1[:], var1[:], EPS, None, OP.add)
            rs1 = _inv_sqrt(nc, st, vpe1, "bn1")
            a1 = st.tile([D, 1], F32, tag="a1")
            nc.vector.tensor_tensor(a1[:], rs1[:], g1, op=OP.mult)
            m1a1 = st.tile([D, 1], F32, tag="m1a1")
            nc.vector.tensor_tensor(m1a1[:], m1[:], a1[:], op=OP.mult)
            bb1 = st.tile([D, 1], F32, tag="bb1")
            nc.vector.tensor_tensor(bb1[:], b1, m1a1[:], op=OP.subtract)

            # ---------- normalized projections ----------
            # P'[d, t] = a1*PTloc + bb1 ; Q'[d, a] = a1*QT
            Pp = cp.tile([D, TLOC], F32, tag="Pp")
            nc.scalar.activation(Pp[:], ptl[:], AF.Identity, bias=bb1[:],
                                 scale=a1[:])

            # copy of PT shard also kept in fp32 for nothing else; skip.
            Qp = cp.tile([D, A], r_dtype, tag="Qp")
            nc.scalar.activation(Qp[:], qt[:], AF.Identity, scale=a1[:])
            w2r = cp.tile([D, 1], r_dtype, tag="w2r")
            nc.vector.tensor_copy(w2r[:], w2)

            # rank-1 linear part: pdot[t] = sum_d w2*P', qdot[a] = sum_d w2*Q'
            pdot_ps = sps.tile([1, TLOC], F32, tag="sm", name="pdot_ps")
            nc.tensor.matmul(pdot_ps[:], w2, Pp[:], start=True, stop=True)
            pdot01 = cp.tile([1, TLOC], F32, tag="pdot01")
            nc.vector.tensor_scalar(pdot01[:], pdot_ps[:], NEG, None, OP.mult)
            pdot_b = cp.tile([D, TLOC], F32, tag="pdot_b")
            nc.gpsimd.partition_broadcast(pdot_b[:], pdot01[:])

            qdot_ps = sps.tile([D, 4], F32, tag="sm", name="qdot_ps")
            for c in range(4):
                nc.tensor.matmul(qdot_ps[:, c:c + 1],
                                 Qp[:, c * 128:(c + 1) * 128], w2r[:],
                                 start=True, stop=True)
            qdot01 = cp.tile([D, 4], F32, tag="qdot01")
            nc.vector.tensor_scalar(qdot01[:], qdot_ps[:], NEG, None, OP.mult)

            # ---------- main loop over local tasks ----------
            racc = [rps.tile([D, TLOC], F32, tag=f"racc{c}", name=f"racc{c}")
                    for c in range(4)]
            for t in range(TLOC):
                use_pool = t % 6 == 4
                r = rb.tile([D, A], r_dtype, tag="rp" if use_pool else "r",
                            bufs=4 if use_pool else 8, name="r")
                bias = Pp[:, t:t + 1]
                if use_pool:
                    nc.gpsimd.tensor_scalar(r[:], Qp[:], bias, 0.0, OP.add, OP.max)
                elif t % 6 == 1:
                    nc.scalar.activation(r[:], Qp[:], AF.Relu, bias=bias)
                else:
                    nc.vector.tensor_scalar(r[:], Qp[:], bias, 0.0, OP.add, OP.max)
                for c in range(4):
                    nc.tensor.matmul(racc[c][:, t:t + 1],
                                     r[:, c * 128:(c + 1) * 128], w2r[:],
                                     start=True, stop=True)

            # pdqb[c] = 0.01*pdot (bcast) + 0.01*qdot[c]  — ready early,
            # overlaps the main loop
            pdqb = []
            for c in range(4):
                pq = cp.tile([D, TLOC], F32, tag=f"pdqb{c}", name=f"pdqb{c}")
                nc.vector.tensor_scalar(pq[:], pdot_b[:], qdot01[:, c:c + 1],
                                        None, OP.add)
                pdqb.append(pq)

            # ---------- s_pre = 0.99*R + pdqb; fused row-sums ----------
            ssum = cp.tile([D, 4], F32, tag="ssum")
            ssq = cp.tile([D, 4], F32, tag="ssq")
            s_scr = wk.tile([D, TLOC], F32, tag="s_scr")
            spre = []
            for c in range(4):
                sp = cp.tile([D, TLOC], F32, tag=f"spre{c}")
                nc.vector.scalar_tensor_tensor(sp[:], racc[c][:], 1.0 - NEG,
                                               pdqb[c][:], OP.mult, OP.add,
                                               accum_out=ssum[:, c:c + 1])
                spre.append(sp)
                nc.scalar.activation(s_scr[:], sp[:], AF.Square,
                                     accum_out=ssq[:, c:c + 1])
            stats8 = cp.tile([D, 8], F32, tag="stats8")
            nc.vector.memset(stats8[:], 0.0)
            nc.vector.tensor_reduce(stats8[:, 0:1], ssum[:], axis=AX.X, op=OP.add)
            nc.vector.tensor_reduce(stats8[:, 1:2], ssq[:], axis=AX.X, op=OP.add)
            ones = cp.tile([D, 1], F32, tag="ones")
            nc.vector.memset(ones[:], 1.0)
            stat_ps = sps.tile([1, 8], F32, tag="sm", name="stat_ps")
            nc.tensor.matmul(stat_ps[:], ones[:], stats8[:], start=True, stop=True)
            cc_sb = cp.tile([1, 8], F32, tag="cc_sb")
            nc.vector.tensor_copy(cc_sb[:], stat_ps[:])
            cc_in = dp.tile([1, 8], F32, tag="cc_in")
            cc_out = dp.tile([NCORES, 8], F32, tag="cc_out")
            nc.sync.dma_start(out=cc_in[:], in_=cc_sb[:])
            if single:
                for rr in range(NCORES):
                    nc.sync.dma_start(out=cc_out[rr:rr + 1, :], in_=cc_in[:])
            else:
                nc.gpsimd.collective_compute(
                    "AllGather", OP.bypass, replica_groups=[list(range(NCORES))],
                    ins=[cc_in.opt()], outs=[cc_out.opt()],
                )
            agb = cp.tile([D, NCORES * 8], F32, tag="agb")
            nc.sync.dma_start(
                out=agb[:],
                in_=cc_out[:].rearrange("a b -> (a b)").partition_broadcast(D))

            # ---------- global BN2 scalars (replicated on 128 partitions) ----
            t32 = cp.tile([D, 32], F32, tag="t32")
            nc.vector.tensor_tensor(t32[:], agb[:, 0:32], agb[:, 32:64], op=OP.add)
            t16 = cp.tile([D, 16], F32, tag="t16")
            nc.vector.tensor_tensor(t16[:], t32[:, 0:16], t32[:, 16:32], op=OP.add)
            tot8 = cp.tile([D, 8], F32, tag="tot8")
            nc.vector.tensor_tensor(tot8[:], t16[:, 0:8], t16[:, 8:16], op=OP.add)
            m2 = cp.tile([D, 1], F32, tag="m2")
            nc.vector.tensor_scalar(m2[:], tot8[:, 0:1], 1.0 / (T * A), None, OP.mult)
            e2 = cp.tile([D, 1], F32, tag="e2")
            nc.vector.tensor_scalar(e2[:], tot8[:, 1:2], 1.0 / (T * A), None, OP.mult)
            m2sq = cp.tile([D, 1], F32, tag="m2sq")
            nc.vector.tensor_tensor(m2sq[:], m2[:], m2[:], op=OP.mult)
            v2 = cp.tile([D, 1], F32, tag="v2")
            nc.vector.tensor_tensor(v2[:], e2[:], m2sq[:], op=OP.subtract)
            vpe2 = cp.tile([D, 1], F32, tag="vpe2")
            nc.vector.tensor_scalar(vpe2[:], v2[:], EPS, None, OP.add)
            rs2 = _inv_sqrt(nc, cp, vpe2, "bn2")
            g2col = cp.tile([D, 2], F32, tag="g2col")
            nc.gpsimd.partition_broadcast(g2col[:], g2b2)
            c_col = cp.tile([D, 1], F32, tag="c_col")
            nc.vector.tensor_tensor(c_col[:], rs2[:], g2col[:, 0:1], op=OP.mult)
            cm2 = cp.tile([D, 1], F32, tag="cm2")
            nc.vector.tensor_tensor(cm2[:], c_col[:], m2[:], op=OP.mult)
            d_col = cp.tile([D, 1], F32, tag="d_col")
            nc.vector.tensor_tensor(d_col[:], g2col[:, 1:2], cm2[:], op=OP.subtract)
            # mask+offset row tile: mskd_b = msk_b + d_col
            mskd_b = cp.tile([D, TLOC], F32, tag="mskd_b")
            nc.vector.tensor_scalar(mskd_b[:], msk_b[:], d_col[:], None, OP.add)

            # ---------- finalize + store (single DMA, 3-D AP) ----------
            ob = cp.tile([D, 4 * TLOC], F32, tag="ob")
            for c in range(4):
                nc.vector.scalar_tensor_tensor(ob[:, c * TLOC:(c + 1) * TLOC],
                                               spre[c][:], c_col[:],
                                               mskd_b[:], OP.mult, OP.add)
            nc.sync.dma_start(
                out=pol_d.ap().rearrange("(c p) t -> p c t", c=4),
                in_=ob[:].rearrange("p (c t) -> p c t", c=4))

            if dbg:
                rc0 = cp.tile([D, TLOC], F32, tag="dbg_rc0")
                nc.vector.tensor_copy(rc0[:], racc[0][:])
                nc.sync.dma_start(out=dbg_racc0_d.ap(), in_=rc0[:])
                nc.sync.dma_start(out=dbg_spre0_d.ap(), in_=spre[0][:])
                nc.sync.dma_start(out=dbg_ag_d.ap(), in_=ag_sb[:])
                dcols = cp.tile([D, 12], F32, tag="dbg_cols")
                for i, col in enumerate([m2, e2, v2, vpe2, rs2, c_col, d_col,
                                         a1, bb1, var1, rs1, pdot_b[:, 0:1]]):
                    nc.vector.tensor_copy(dcols[:, i:i + 1], col if isinstance(col, bass.AP) else col[:])
                nc.sync.dma_start(out=dbg_cols_d.ap(), in_=dcols[:])
                dst_t = cp.tile([D, 16], F32, tag="dbg_st")
                nc.vector.tensor_copy(dst_t[:, 0:4], ssum[:])
                nc.vector.tensor_copy(dst_t[:, 4:8], ssq[:])
                nc.vector.tensor_copy(dst_t[:, 8:16], stats8[:])
                nc.sync.dma_start(out=dbg_st_d.ap(), in_=dst_t[:])
                dcc = cp.tile([1, 16], F32, tag="dbg_cc")
                nc.vector.tensor_copy(dcc[:, 0:8], cc_sb[:])
                nc.vector.tensor_copy(dcc[:, 8:16], stat_ps[:])
                nc.sync.dma_start(out=dbg_cc_d.ap(), in_=dcc[:])

    nc.compile()
    return nc


def _get_nc():
    key = (R_DTYPE, ACT_EVERY)
    if key not in _CACHE:
        _CACHE[key] = _build()
    return _CACHE[key]


def prep_in_maps(inputs) -> list:
    nf_task = np.asarray(inputs["nf_task"], dtype=np.float32)
    nf_agent = np.asarray(inputs["nf_agent"], dtype=np.float32)
    W1 = np.asarray(inputs["W1"], dtype=np.float32)
    gamma1 = np.asarray(inputs["gamma1"], dtype=np.float32)
    beta1 = np.asarray(inputs["beta1"], dtype=np.float32)
    W2 = np.asarray(inputs["W2"], dtype=np.float32)
    gamma2 = np.asarray(inputs["gamma2"], dtype=np.float32)
    beta2 = np.asarray(inputs["beta2"], dtype=np.float32)
    finished = np.asarray(inputs["finished"])

    taskT = np.ascontiguousarray(nf_task.T).astype(np.float16)    # [D, T]
    agentT = np.ascontiguousarray(nf_agent.T).astype(np.float16)  # [D, A]
    w1srcT = np.ascontiguousarray(W1[:, :D].T).astype(np.float16)
    w1dstT = np.ascontiguousarray(W1[:, D:].T).astype(np.float16)
    gbw = np.stack([gamma1, beta1, W2.reshape(-1)], axis=1).astype(np.float32)
    mskfull = np.where(finished.astype(bool), -np.inf, 0.0).astype(np.float32)

    in_maps = []
    for c in range(NCORES):
        sl = slice(c * TLOC, (c + 1) * TLOC)
        meta = np.concatenate([[gamma2.reshape(-1)[0], beta2.reshape(-1)[0]],
                               mskfull[sl]]).astype(np.float32).reshape(1, -1)
        w16 = np.concatenate([w1srcT, w1dstT, taskT[:, sl]], axis=1)
        in_maps.append({
            "taskT": taskT,
            "agentT": agentT,
            "w16": np.ascontiguousarray(w16),
            "gbw": gbw,
            "meta": meta,
        })
    return in_maps


def kernel(**inputs) -> np.ndarray:
    in_maps = prep_in_maps(inputs)
    nc = _get_nc()
    res = bass_utils.run_bass_kernel_spmd(nc, in_maps, core_ids=list(range(NCORES)))
    return np.concatenate([res.results[c]["policy"] for c in range(NCORES)], axis=1)


if __name__ == "__main__":
    # quick self-exercise with random data
    rng = np.random.default_rng(0)
    ins = {
        "nf_task": rng.standard_normal((T, D)).astype(np.float32),
        "nf_agent": rng.standard_normal((A, D)).astype(np.float32),
        "W1": (rng.standard_normal((D, 2 * D)) * 0.05).astype(np.float32),
        "gamma1": np.ones(D, np.float32),
        "beta1": np.zeros(D, np.float32),
        "W2": (rng.standard_normal((1, D)) * 0.05).astype(np.float32),
        "gamma2": np.ones(1, np.float32),
        "beta2": np.zeros(1, np.float32),
        "finished": rng.integers(0, 2, T).astype(np.int32),
    }
    out = kernel(**ins)
    print("out", out.shape, out.dtype, np.isneginf(out).sum())


# revision 56
# speedup vs baseline: 1.0169x; 1.0169x over previous
"""Trainium2 Bass kernel for nn_Bipartite: bipartite GNN edge scoring.

reference computation:
    src = nf_task @ W1[:, :D].T          [T, D]
    dst = nf_agent @ W1[:, D:].T         [A, D]
    h[t,a,:] = BN1(src[t] + dst[a]); h = lrelu(h)
    s[t,a] = h @ W2[0];  s = BN2(s); s[finished] = -inf;  out = s.T  [A, T]

Key identities used (avoid materializing [T, A, D]):
  - BN1 batch stats decompose: mean = mean_t(src) + mean_a(dst),
    var = var_t(src) + var_a(dst)   (h is a broadcast sum).
  - lrelu(x) = 0.01*x + 0.99*relu(x), and the 0.01*x part of the score is
    rank-1:  sum_d w2_d * x[t,a,d] = pdot[t] + qdot[a].
  - so  s_raw[t,a] = 0.99 * sum_d w2_d * relu(P'[d,t] + Q'[d,a])
                   + 0.01 * (pdot[t] + qdot[a])
    with P' = a1*src.T + b1 (BN1 affine folded task-side), Q' = a1*dst.T.

Sharding: tasks (T axis) split across 8 cores; BN1 stats computed
replicated on every core (cheap); BN2 batch stats via a tiny AllGather of
per-core (sum, sumsq); the -inf mask and final affine are applied on-device.
Per-task inner loop: one fused add+relu op (DVE or ACT) produces
r = relu(Q' + P'[:,t]) [D=128 part, A=512 free]; four matmuls with r-chunks
stationary x w2 column moving write score columns directly into PSUM in
[agent, task] (output-transposed) layout.
"""
import os
import sys

_REPO = "/opt/trn_rl_repo"
if not os.path.isdir(_REPO):
    _REPO = "/root/.axon_site/_ro/trn_rl_repo"
if _REPO in sys.path:
    sys.path.remove(_REPO)
sys.path.insert(0, _REPO)

import numpy as np
import concourse.bass as bass  # noqa: F401  (registers engines)
import concourse.bacc as bacc
import concourse.tile as tile
import concourse.mybir as mybir
from concourse import bass_utils

T, A, D = 1024, 512, 128
NCORES = 8
TLOC = T // NCORES  # 128 tasks per core
EPS = 1e-5
NEG = 0.01  # LeakyReLU slope
F32 = mybir.dt.float32
BF16 = mybir.dt.bfloat16
AX = mybir.AxisListType
OP = mybir.AluOpType
AF = mybir.ActivationFunctionType

# r-path dtype: fp16 halves PE stationary-load time and doubles DVE rate
# vs fp32 (same speed as bf16 but 8x better mantissa; values are < ~10 so
# fp16 range is ample).
F16 = mybir.dt.float16
R_DTYPE = F16
ACT_EVERY = 4  # every ACT_EVERY-th task runs on ScalarE, rest on VectorE

_CACHE: dict = {}


def _inv_sqrt(nc, pool, v, name):
    """[128,1] column: 1/sqrt(v) = exp(-0.5*ln(v)), one Newton step.

    Uses Ln+Exp (one ACT table set, shared with Relu/Square — avoids the
    ~2.6us extra table load that Sqrt's set would cost), then a
    Newton-Raphson step y*(1.5 - 0.5*v*y^2) on the accurate DVE ops.
    """
    # quake seed: bits = MAGIC - (bits(v) >> 1), done as
    # ((v>>1) xor 0xffffffff) - (0xffffffff - MAGIC)   (all uint32 ALU)
    U32 = mybir.dt.uint32
    MAGIC = 0x5F3759DF
    K2 = 0xFFFFFFFF - MAGIC
    tA = pool.tile([D, 1], F32, tag=f"{name}_tA")
    nc.vector.tensor_scalar(tA[:].bitcast(U32), v[:].bitcast(U32), 1, None,
                            OP.logical_shift_right)
    tB = pool.tile([D, 1], F32, tag=f"{name}_tB")
    nc.vector.tensor_scalar(tB[:].bitcast(U32), tA[:].bitcast(U32),
                            0xFFFFFFFF, None, OP.bitwise_xor)
    y = pool.tile([D, 1], F32, tag=f"{name}_y0")
    nc.vector.tensor_scalar(y[:].bitcast(U32), tB[:].bitcast(U32),
                            K2, None, OP.subtract)
    # three Newton-Raphson steps: y <- y*(1.5 - 0.5*v*y^2)
    for it in range(2):
        y2 = pool.tile([D, 1], F32, tag=f"{name}_y2{it}")
        nc.vector.tensor_tensor(y2[:], y[:], y[:], op=OP.mult)
        vy2 = pool.tile([D, 1], F32, tag=f"{name}_vy2{it}")
        nc.vector.tensor_tensor(vy2[:], v[:], y2[:], op=OP.mult)
        h = pool.tile([D, 1], F32, tag=f"{name}_h{it}")
        nc.vector.tensor_scalar(h[:], vy2[:], -0.5, 1.5, OP.mult, OP.add)
        yn = pool.tile([D, 1], F32, tag=f"{name}_yn{it}")
        nc.vector.tensor_tensor(yn[:], y[:], h[:], op=OP.mult)
        y = yn
    return y


def _build(r_dtype=R_DTYPE, act_every=ACT_EVERY, dbg=False, single=False):
    """single=True: 1-core variant with the collective replaced by a DRAM
    bounce copy — numerically wrong stats but timing-shape identical; used
    for TimelineSim cost-model benchmarking (which is single-core only)."""
    nc = bacc.Bacc("TRN2", target_bir_lowering=False, debug=False,
                   enable_asserts=True, num_devices=1 if single else NCORES)

    # ---- kernel I/O (per core) ----
    taskT_d = nc.dram_tensor("taskT", [D, T], F16, kind="ExternalInput")
    agentT_d = nc.dram_tensor("agentT", [D, A], F16, kind="ExternalInput")
    w16_d = nc.dram_tensor("w16", [D, 2 * D + TLOC + 2], F16, kind="ExternalInput")
    gbw_d = nc.dram_tensor("gbw", [D, 3], F32, kind="ExternalInput")
    meta_d = nc.dram_tensor("meta", [1, 2 + TLOC], F32, kind="ExternalInput")
    pol_d = nc.dram_tensor("policy", [A, TLOC], F32, kind="ExternalOutput")
    if dbg:
        dbg_racc0_d = nc.dram_tensor("dbg_racc0", [D, TLOC], F32, kind="ExternalOutput")
        dbg_spre0_d = nc.dram_tensor("dbg_spre0", [D, TLOC], F32, kind="ExternalOutput")
        dbg_ag_d = nc.dram_tensor("dbg_ag", [1, NCORES * 8], F32, kind="ExternalOutput")
        dbg_cols_d = nc.dram_tensor("dbg_cols", [D, 12], F32, kind="ExternalOutput")
        dbg_st_d = nc.dram_tensor("dbg_st", [D, 16], F32, kind="ExternalOutput")
        dbg_cc_d = nc.dram_tensor("dbg_cc", [1, 16], F32, kind="ExternalOutput")

    with tile.TileContext(nc) as tc:
        with tc.tile_pool(name="const", bufs=1) as cp, \
             tc.tile_pool(name="work", bufs=2) as wk, \
             tc.tile_pool(name="rbuf", bufs=10) as rb, \
             tc.tile_pool(name="proj_ps", bufs=3, space="PSUM") as pps, \
             tc.tile_pool(name="small_ps", bufs=1, space="PSUM") as sps, \
             tc.tile_pool(name="racc_ps", bufs=1, space="PSUM") as rps, \
             tc.tile_pool(name="dram", bufs=1, space="DRAM") as dp:

            # ---------- loads ----------
            tT = cp.tile([D, T], F16, tag="tT")
            for _q in range(2):
                nc.sync.dma_start(out=tT[:, _q * 512:(_q + 1) * 512],
                                  in_=taskT_d[:, _q * 512:(_q + 1) * 512])
            aT = cp.tile([D, A], F16, tag="aT")
            nc.sync.dma_start(out=aT[:], in_=agentT_d.ap())
            w16 = cp.tile([D, 2 * D + TLOC + 2], F16, tag="w16")
            nc.sync.dma_start(out=w16[:], in_=w16_d.ap())
            wsrc, wdst = w16[:, 0:D], w16[:, D:2 * D]
            tTl_ext = w16[:, 2 * D:2 * D + TLOC + 1]   # shard + task-sum col
            asum_col = w16[:, 2 * D + TLOC + 1:]
            gbw = cp.tile([D, 3], F32, tag="gbw")
            nc.sync.dma_start(out=gbw[:], in_=gbw_d.ap())
            g1, b1, w2 = gbw[:, 0:1], gbw[:, 1:2], gbw[:, 2:3]
            meta = cp.tile([1, 2 + TLOC], F32, tag="meta")
            nc.sync.dma_start(out=meta[:], in_=meta_d.ap())
            g2b2 = meta[:, 0:2]
            mskr = meta[:, 2:]

            # mask broadcast to all partitions (needed only at the end;
            # no data deps so it can schedule any time)
            msk_b = cp.tile([D, TLOC], F32, tag="msk_b")
            nc.gpsimd.partition_broadcast(msk_b[:], mskr)

            # ---------- projections (PE) + BN1 stats ----------
            # PT[j, t] = sum_c W1src[j, c] * taskT[c, t]  (full T for stats)
            sqcols = cp.tile([D, 6], F32, tag="sqcols")     # sqsums

            sq_scr = wk.tile([D, 512], F32, tag="sq_scr")
            sq_scr2 = wk.tile([D, 512], F32, tag="sq_scr2")

            def stats_of(ps_tile, n, col):
                nc.scalar.activation(sq_scr2[:, 0:n], ps_tile[:, 0:n], AF.Square,
                                     accum_out=sqcols[:, col:col + 1])

            for half in range(2):
                pt = pps.tile([D, 512], F32, tag="proj")
                nc.tensor.matmul(pt[:], wsrc, tT[:, half * 512:(half + 1) * 512],
                                 start=True, stop=True)
                stats_of(pt, 512, half)
            qt = pps.tile([D, 512], F32, tag="proj")
            nc.tensor.matmul(qt[:], wdst, aT[:], start=True, stop=True)
            stats_of(qt, 512, 2)
            ptl = pps.tile([D, TLOC + 2], F32, tag="proj", name="ptl")
            nc.tensor.matmul(ptl[:, 0:TLOC + 1], wsrc, tTl_ext,
                             start=True, stop=True)
            nc.tensor.matmul(ptl[:, TLOC + 1:], wdst, asum_col,
                             start=True, stop=True)
            sumP = ptl[:, TLOC:TLOC + 1]    # W1src @ sum_t(x)  = sum_t PT
            sumQ = ptl[:, TLOC + 1:]        # W1dst @ sum_a(x)  = sum_a QT

            # per-channel BN1 stats [D, 1]
            st = cp  # alias: small stat tiles live in const pool
            meanP = st.tile([D, 1], F32, tag="meanP")
            nc.vector.tensor_scalar(meanP[:], sumP, 1.0 / T, None, OP.mult)
            meanQ = st.tile([D, 1], F32, tag="meanQ")
            nc.vector.tensor_scalar(meanQ[:], sumQ, 1.0 / A, None, OP.mult)
            m1 = st.tile([D, 1], F32, tag="m1")
            nc.vector.tensor_tensor(m1[:], meanP[:], meanQ[:], op=OP.add)
            # E[P^2] + E[Q^2]
            sqP = st.tile([D, 1], F32, tag="sqP")
            nc.vector.tensor_tensor(sqP[:], sqcols[:, 0:1], sqcols[:, 1:2], op=OP.add)
            ex2 = st.tile([D, 1], F32, tag="ex2")
            nc.vector.tensor_scalar(ex2[:], sqP[:], 1.0 / T, None, OP.mult)
            ex2q = st.tile([D, 1], F32, tag="ex2q")
            nc.vector.tensor_scalar(ex2q[:], sqcols[:, 2:3], 1.0 / A, None, OP.mult)
            # var = ex2 - meanP^2 + ex2q - meanQ^2
            mP2 = st.tile([D, 1], F32, tag="mP2")
            nc.vector.tensor_tensor(mP2[:], meanP[:], meanP[:], op=OP.mult)
            mQ2 = st.tile([D, 1], F32, tag="mQ2")
            nc.vector.tensor_tensor(mQ2[:], meanQ[:], meanQ[:], op=OP.mult)
            v_a = st.tile([D, 1], F32, tag="v_a")
            nc.vector.tensor_tensor(v_a[:], ex2[:], mP2[:], op=OP.subtract)
            v_b = st.tile([D, 1], F32, tag="v_b")
            nc.vector.tensor_tensor(v_b[:], ex2q[:], mQ2[:], op=OP.subtract)
            var1 = st.tile([D, 1], F32, tag="var1")
            nc.vector.tensor_tensor(var1[:], v_a[:], v_b[:], op=OP.add)
            vpe1 = st.tile([D, 1], F32, tag="vpe1")
            nc.vector.tensor_scalar(vpe1[:], var1[:], EPS, None, OP.add)
            rs1 = _inv_sqrt(nc, st, vpe1, "bn1")
            a1 = st.tile([D, 1], F32, tag="a1")
            nc.vector.tensor_tensor(a1[:], rs1[:], g1, op=OP.mult)
            m1a1 = st.tile([D, 1], F32, tag="m1a1")
            nc.vector.tensor_tensor(m1a1[:], m1[:], a1[:], op=OP.mult)
            bb1 = st.tile([D, 1], F32, tag="bb1")
            nc.vector.tensor_tensor(bb1[:], b1, m1a1[:], op=OP.subtract)

            # ---------- normalized projections ----------
            # P'[d, t] = a1*PTloc + bb1 ; Q'[d, a] = a1*QT
            Pp = cp.tile([D, TLOC], F32, tag="Pp")
            nc.scalar.activation(Pp[:], ptl[:, 0:TLOC], AF.Identity, bias=bb1[:],
                                 scale=a1[:])

            # copy of PT shard also kept in fp32 for nothing else; skip.
            Qp = cp.tile([D, A], r_dtype, tag="Qp")
            nc.scalar.activation(Qp[:], qt[:], AF.Identity, scale=a1[:])
            w2r = cp.tile([D, 1], r_dtype, tag="w2r")
            nc.vector.tensor_copy(w2r[:], w2)

            # rank-1 linear part: pdot[t] = sum_d w2*P', qdot[a] = sum_d w2*Q'
            pdot_ps = sps.tile([1, TLOC], F32, tag="sm", name="pdot_ps")
            nc.tensor.matmul(pdot_ps[:], w2, Pp[:], start=True, stop=True)
            pdot01 = cp.tile([1, TLOC], F32, tag="pdot01")
            nc.vector.tensor_scalar(pdot01[:], pdot_ps[:], NEG, None, OP.mult)
            pdot_b = cp.tile([D, TLOC], F32, tag="pdot_b")
            nc.gpsimd.partition_broadcast(pdot_b[:], pdot01[:])

            qdot_ps = sps.tile([D, 4], F32, tag="sm", name="qdot_ps")
            for c in range(4):
                nc.tensor.matmul(qdot_ps[:, c:c + 1],
                                 Qp[:, c * 128:(c + 1) * 128], w2r[:],
                                 start=True, stop=True)
            qdot01 = cp.tile([D, 4], F32, tag="qdot01")
            nc.vector.tensor_scalar(qdot01[:], qdot_ps[:], NEG, None, OP.mult)

            # ---------- main loop over local tasks ----------
            racc = [rps.tile([D, TLOC], F32, tag=f"racc{c}", name=f"racc{c}")
                    for c in range(4)]
            for t in range(TLOC):
                use_pool = t % 6 == 4
                r = rb.tile([D, A], r_dtype, tag="rp" if use_pool else "r",
                            bufs=4 if use_pool else 8, name="r")
                bias = Pp[:, t:t + 1]
                if use_pool:
                    nc.gpsimd.tensor_scalar(r[:], Qp[:], bias, 0.0, OP.add, OP.max)
                elif t % 6 == 1:
                    nc.scalar.activation(r[:], Qp[:], AF.Relu, bias=bias)
                else:
                    nc.vector.tensor_scalar(r[:], Qp[:], bias, 0.0, OP.add, OP.max)
                for c in range(4):
                    nc.tensor.matmul(racc[c][:, t:t + 1],
                                     r[:, c * 128:(c + 1) * 128], w2r[:],
                                     start=True, stop=True)

            # pdqb[c] = 0.01*pdot (bcast) + 0.01*qdot[c]  — ready early,
            # overlaps the main loop
            pdqb = []
            for c in range(4):
                pq = cp.tile([D, TLOC], F32, tag=f"pdqb{c}", name=f"pdqb{c}")
                nc.vector.tensor_scalar(pq[:], pdot_b[:], qdot01[:, c:c + 1],
                                        None, OP.add)
                pdqb.append(pq)

            # ---------- s_pre = 0.99*R + pdqb; fused row-sums ----------
            ssum = cp.tile([D, 4], F32, tag="ssum")
            ssq = cp.tile([D, 4], F32, tag="ssq")
            s_scr = wk.tile([D, TLOC], F32, tag="s_scr")
            spre = []
            for c in range(4):
                sp = cp.tile([D, TLOC], F32, tag=f"spre{c}")
                nc.vector.scalar_tensor_tensor(sp[:], racc[c][:], 1.0 - NEG,
                                               pdqb[c][:], OP.mult, OP.add,
                                               accum_out=ssum[:, c:c + 1])
                spre.append(sp)
                nc.scalar.activation(s_scr[:], sp[:], AF.Square,
                                     accum_out=ssq[:, c:c + 1])
            stats8 = cp.tile([D, 8], F32, tag="stats8")
            nc.vector.memset(stats8[:], 0.0)
            nc.vector.tensor_reduce(stats8[:, 0:1], ssum[:], axis=AX.X, op=OP.add)
            nc.vector.tensor_reduce(stats8[:, 1:2], ssq[:], axis=AX.X, op=OP.add)
            ones = cp.tile([D, 1], F32, tag="ones")
            nc.vector.memset(ones[:], 1.0)
            stat_ps = sps.tile([1, 8], F32, tag="sm", name="stat_ps")
            nc.tensor.matmul(stat_ps[:], ones[:], stats8[:], start=True, stop=True)
            cc_sb = cp.tile([1, 8], F32, tag="cc_sb")
            nc.vector.tensor_copy(cc_sb[:], stat_ps[:])
            cc_in = dp.tile([1, 8], F32, tag="cc_in")
            cc_out = dp.tile([NCORES, 8], F32, tag="cc_out")
            nc.sync.dma_start(out=cc_in[:], in_=cc_sb[:])
            if single:
                for rr in range(NCORES):
                    nc.sync.dma_start(out=cc_out[rr:rr + 1, :], in_=cc_in[:])
            else:
                nc.gpsimd.collective_compute(
                    "AllGather", OP.bypass, replica_groups=[list(range(NCORES))],
                    ins=[cc_in.opt()], outs=[cc_out.opt()],
                )
            agb = cp.tile([D, NCORES * 8], F32, tag="agb")
            nc.sync.dma_start(
                out=agb[:],
                in_=cc_out[:].rearrange("a b -> (a b)").partition_broadcast(D))

            # ---------- global BN2 scalars (replicated on 128 partitions) ----
            t32 = cp.tile([D, 32], F32, tag="t32")
            nc.vector.tensor_tensor(t32[:], agb[:, 0:32], agb[:, 32:64], op=OP.add)
            t16 = cp.tile([D, 16], F32, tag="t16")
            nc.vector.tensor_tensor(t16[:], t32[:, 0:16], t32[:, 16:32], op=OP.add)
            tot8 = cp.tile([D, 8], F32, tag="tot8")
            nc.vector.tensor_tensor(tot8[:], t16[:, 0:8], t16[:, 8:16], op=OP.add)
            m2 = cp.tile([D, 1], F32, tag="m2")
            nc.vector.tensor_scalar(m2[:], tot8[:, 0:1], 1.0 / (T * A), None, OP.mult)
            e2 = cp.tile([D, 1], F32, tag="e2")
            nc.vector.tensor_scalar(e2[:], tot8[:, 1:2], 1.0 / (T * A), None, OP.mult)
            m2sq = cp.tile([D, 1], F32, tag="m2sq")
            nc.vector.tensor_tensor(m2sq[:], m2[:], m2[:], op=OP.mult)
            v2 = cp.tile([D, 1], F32, tag="v2")
            nc.vector.tensor_tensor(v2[:], e2[:], m2sq[:], op=OP.subtract)
            vpe2 = cp.tile([D, 1], F32, tag="vpe2")
            nc.vector.tensor_scalar(vpe2[:], v2[:], EPS, None, OP.add)
            rs2 = _inv_sqrt(nc, cp, vpe2, "bn2")
            g2col = cp.tile([D, 2], F32, tag="g2col")
            nc.gpsimd.partition_broadcast(g2col[:], g2b2)
            c_col = cp.tile([D, 1], F32, tag="c_col")
            nc.vector.tensor_tensor(c_col[:], rs2[:], g2col[:, 0:1], op=OP.mult)
            cm2 = cp.tile([D, 1], F32, tag="cm2")
            nc.vector.tensor_tensor(cm2[:], c_col[:], m2[:], op=OP.mult)
            d_col = cp.tile([D, 1], F32, tag="d_col")
            nc.vector.tensor_tensor(d_col[:], g2col[:, 1:2], cm2[:], op=OP.subtract)
            # mask+offset row tile: mskd_b = msk_b + d_col
            mskd_b = cp.tile([D, TLOC], F32, tag="mskd_b")
            nc.vector.tensor_scalar(mskd_b[:], msk_b[:], d_col[:], None, OP.add)

            # ---------- finalize + store (single DMA, 3-D AP) ----------
            ob = cp.tile([D, 4 * TLOC], F32, tag="ob")
            for c in range(4):
                nc.vector.scalar_tensor_tensor(ob[:, c * TLOC:(c + 1) * TLOC],
                                               spre[c][:], c_col[:],
                                               mskd_b[:], OP.mult, OP.add)
            nc.sync.dma_start(
                out=pol_d.ap().rearrange("(c p) t -> p c t", c=4),
                in_=ob[:].rearrange("p (c t) -> p c t", c=4))

            if dbg:
                rc0 = cp.tile([D, TLOC], F32, tag="dbg_rc0")
                nc.vector.tensor_copy(rc0[:], racc[0][:])
                nc.sync.dma_start(out=dbg_racc0_d.ap(), in_=rc0[:])
                nc.sync.dma_start(out=dbg_spre0_d.ap(), in_=spre[0][:])
                nc.sync.dma_start(out=dbg_ag_d.ap(), in_=ag_sb[:])
                dcols = cp.tile([D, 12], F32, tag="dbg_cols")
                for i, col in enumerate([m2, e2, v2, vpe2, rs2, c_col, d_col,
                                         a1, bb1, var1, rs1, pdot_b[:, 0:1]]):
                    nc.vector.tensor_copy(dcols[:, i:i + 1], col if isinstance(col, bass.AP) else col[:])
                nc.sync.dma_start(out=dbg_cols_d.ap(), in_=dcols[:])
                dst_t = cp.tile([D, 16], F32, tag="dbg_st")
                nc.vector.tensor_copy(dst_t[:, 0:4], ssum[:])
                nc.vector.tensor_copy(dst_t[:, 4:8], ssq[:])
                nc.vector.tensor_copy(dst_t[:, 8:16], stats8[:])
                nc.sync.dma_start(out=dbg_st_d.ap(), in_=dst_t[:])
                dcc = cp.tile([1, 16], F32, tag="dbg_cc")
                nc.vector.tensor_copy(dcc[:, 0:8], cc_sb[:])
                nc.vector.tensor_copy(dcc[:, 8:16], stat_ps[:])
                nc.sync.dma_start(out=dbg_cc_d.ap(), in_=dcc[:])

    nc.compile()
    return nc


def _get_nc():
    key = (R_DTYPE, ACT_EVERY)
    if key not in _CACHE:
        _CACHE[key] = _build()
    return _CACHE[key]


def prep_in_maps(inputs) -> list:
    nf_task = np.asarray(inputs["nf_task"], dtype=np.float32)
    nf_agent = np.asarray(inputs["nf_agent"], dtype=np.float32)
    W1 = np.asarray(inputs["W1"], dtype=np.float32)
    gamma1 = np.asarray(inputs["gamma1"], dtype=np.float32)
    beta1 = np.asarray(inputs["beta1"], dtype=np.float32)
    W2 = np.asarray(inputs["W2"], dtype=np.float32)
    gamma2 = np.asarray(inputs["gamma2"], dtype=np.float32)
    beta2 = np.asarray(inputs["beta2"], dtype=np.float32)
    finished = np.asarray(inputs["finished"])

    taskT = np.ascontiguousarray(nf_task.T).astype(np.float16)    # [D, T]
    agentT = np.ascontiguousarray(nf_agent.T).astype(np.float16)  # [D, A]
    w1srcT = np.ascontiguousarray(W1[:, :D].T).astype(np.float16)
    w1dstT = np.ascontiguousarray(W1[:, D:].T).astype(np.float16)
    gbw = np.stack([gamma1, beta1, W2.reshape(-1)], axis=1).astype(np.float32)
    mskfull = np.where(finished.astype(bool), -np.inf, 0.0).astype(np.float32)

    in_maps = []
    for c in range(NCORES):
        sl = slice(c * TLOC, (c + 1) * TLOC)
        meta = np.concatenate([[gamma2.reshape(-1)[0], beta2.reshape(-1)[0]],
                               mskfull[sl]]).astype(np.float32).reshape(1, -1)
        tsum = taskT.astype(np.float32).sum(axis=1, keepdims=True).astype(np.float16)
        asum = agentT.astype(np.float32).sum(axis=1, keepdims=True).astype(np.float16)
        w16 = np.concatenate([w1srcT, w1dstT, taskT[:, sl], tsum, asum], axis=1)
        in_maps.append({
            "taskT": taskT,
            "agentT": agentT,
            "w16": np.ascontiguousarray(w16),
            "gbw": gbw,
            "meta": meta,
        })
    return in_maps


def kernel(**inputs) -> np.ndarray:
    in_maps = prep_in_maps(inputs)
    nc = _get_nc()
    res = bass_utils.run_bass_kernel_spmd(nc, in_maps, core_ids=list(range(NCORES)))
    return np.concatenate([res.results[c]["policy"] for c in range(NCORES)], axis=1)


if __name__ == "__main__":
    # quick self-exercise with random data
    rng = np.random.default_rng(0)
    ins = {
        "nf_task": rng.standard_normal((T, D)).astype(np.float32),
        "nf_agent": rng.standard_normal((A, D)).astype(np.float32),
        "W1": (rng.standard_normal((D, 2 * D)) * 0.05).astype(np.float32),
        "gamma1": np.ones(D, np.float32),
        "beta1": np.zeros(D, np.float32),
        "W2": (rng.standard_normal((1, D)) * 0.05).astype(np.float32),
        "gamma2": np.ones(1, np.float32),
        "beta2": np.zeros(1, np.float32),
        "finished": rng.integers(0, 2, T).astype(np.int32),
    }
    out = kernel(**ins)
    print("out", out.shape, out.dtype, np.isneginf(out).sum())
